# revision 1
# baseline (speedup 1.0000x reference)
# Multi-head attention (B=4, S=2048, D=1024, H=16, causal) on 8 Trainium2
# NeuronCores via Bass/Tile.
#
# Sharding: 8 cores = 4 batches x 2 head-groups (8 heads each).
# Each core computes, for its (batch, head-group):
#   qT = Wq_g' @ x_q^T + bq_g'  [F=512 feats x S]  (Wq' pre-scaled by 1/sqrt(dk))
#   kT = Wk_g  @ x_k^T          [F x S]
#   v  = x_v @ Wv_g^T           [S x F]  (+ ones column per head -> denominators)
#   per head: scoresT[j,q] = kT_h[:,j] . qT_h[:,q]   (keys on partitions)
#             expT = exp(scoresT); causal chunks zeroed via gpsimd affine_select
#             outT_h[d,q] (+ denom row d=DK) = contraction of v_aug over keys
#             concatT[f,q] = outT_h * (1/denom broadcast via K=1 matmul)
#   partial^T[e,s] = Wo_g contraction over local features
# Host sums the two head-group partials per batch and adds the folded bias.
#
# Bias algebra (exact): k-bias cancels inside softmax (constant per row);
# v-bias and o-bias fold into a host-side output offset since softmax rows
# sum to 1. Only the q-bias is applied on-device.
#
# The boolean mask input is handled generically: each (512-query x 128-key)
# chunk is classified skip / free / causal-affine / arbitrary-pattern; only
# non-skip chunks are computed, affine chunks use gpsimd affine_select, and
# arbitrary mixed patterns are shipped and multiplied on the DVE.

import numpy as np
from contextlib import ExitStack

import concourse.bass as bass
import concourse.bacc as bacc
import concourse.mybir as mybir
import concourse.tile as tile
from concourse.bass_utils import run_bass_kernel_spmd

AF = mybir.ActivationFunctionType
ALU = mybir.AluOpType
F32 = mybir.dt.float32
F32R = mybir.dt.float32r
F16 = mybir.dt.float16

B, S, D, H, DK = 4, 2048, 1024, 16, 64
P, SF = 128, 512          # partition tile / free-dim chunk
NH = 8                    # heads per core
F = NH * DK               # 512 local features
NCO = D // P              # 8 contraction chunks
NFP = F // P              # 4 feature-partition chunks
NSF = S // SF             # 4 seq free chunks
NJJ = S // P              # 16 key chunks
NEC = D // P              # 8 output-feature chunks
SCALE = 1.0 / np.sqrt(DK)

MAX_MASK_PATTERNS = 16


def _body(ctx, tc, xq, xk, xv, wq, wk, wv, wo, bq, mm, ones_d, ones_h,
          outT, classes, n_pat):
    nc = tc.nc
    persist = ctx.enter_context(tc.tile_pool(name="persist", bufs=1))

    bq_sb = persist.tile([P, NFP], F32, tag="bq")
    nc.sync.dma_start(out=bq_sb, in_=bq.rearrange("(a p) -> p a", p=P))

    mask_sb = []
    for i in range(n_pat):
        mt = persist.tile([P, SF], F16, tag=f"mask{i}", name=f"mask{i}")
        nc.sync.dma_start(out=mt, in_=mm[i])
        mask_sb.append(mt)

    ones_sb = persist.tile([P, DK], F32R, tag="ones")
    nc.sync.dma_start(out=ones_sb, in_=ones_d)

    qT = [persist.tile([P, S], F16, tag=f"qT{i}", name=f"qT{i}") for i in range(NFP)]
    kT = [persist.tile([P, S], F16, tag=f"kT{i}", name=f"kT{i}") for i in range(NFP)]
    vA = [persist.tile([P, NH, DK + 1], F16, tag=f"v{i}", name=f"v{i}") for i in range(NJJ)]
    cT = [persist.tile([P, S], F16, tag=f"cT{i}", name=f"cT{i}") for i in range(NFP)]

    xq_r = persist.tile([P, NCO, S], F16, tag="xq_r")
    nc.sync.dma_start(out=xq_r, in_=xq.rearrange("(co p) s -> p co s", p=P))
    xk_r = persist.tile([P, NCO, S], F16, tag="xk_r")
    nc.sync.dma_start(out=xk_r, in_=xk.rearrange("(co p) s -> p co s", p=P))
    wq_sb = persist.tile([P, NCO, F], F16, tag="wq_sb")
    nc.sync.dma_start(out=wq_sb, in_=wq.rearrange("(co p) f -> p co f", p=P))
    wk_sb = persist.tile([P, NCO, F], F16, tag="wk_sb")
    nc.sync.dma_start(out=wk_sb, in_=wk.rearrange("(co p) f -> p co f", p=P))
    wv_sb = persist.tile([P, NCO, F], F16, tag="wv_sb")
    nc.sync.dma_start(out=wv_sb, in_=wv.rearrange("(co p) f -> p co f", p=P))

    used = {qi: [jj for jj in range(NJJ) if classes[(qi, jj)] != "skip"]
            for qi in range(NSF)}

    def trim(qi, jj):
        # first query column of this chunk that any key can see; columns
        # before it are fully masked for causal-affine chunks.
        cl = classes[(qi, jj)]
        if isinstance(cl, tuple) and cl[0] == "affine":
            return max(0, cl[1])
        return 0

    xvr = xv.rearrange("(co p) s -> p co s", p=P)

    with tc.tile_pool(name="xvp", bufs=2) as xvp, \
         tc.tile_pool(name="pjp", bufs=1, space="PSUM") as pj_pool, \
         tc.tile_pool(name="sps", bufs=2, space="PSUM") as sp_pool, \
         tc.tile_pool(name="ops", bufs=2, space="PSUM") as op_pool, \
         tc.tile_pool(name="rps", bufs=1, space="PSUM") as rp_pool, \
         tc.tile_pool(name="et", bufs=6) as ep, \
         tc.tile_pool(name="dn", bufs=3) as dnp, \
         tc.tile_pool(name="rc", bufs=3) as rcp, \
         tc.tile_pool(name="stg", bufs=2) as stgp:

        def emit_vproj(sf):
            # V projection for seq chunk sf (key chunks 4sf..4sf+3)
            xt = xvp.tile([P, NCO, SF], F16, tag="xv")
            nc.sync.dma_start(out=xt, in_=xvr[:, :, sf * SF:(sf + 1) * SF])
            for spl in range(SF // P):
                sp = sf * (SF // P) + spl
                ps = pj_pool.tile([P, F], F32, tag="ps")
                for co in range(NCO):
                    nc.tensor.matmul(
                        ps, xt[:, co, spl * P:(spl + 1) * P], wv_sb[:, co, :],
                        start=(co == 0), stop=(co == NCO - 1))
                nc.vector.tensor_copy(out=vA[sp][:, :, 0:DK],
                                      in_=ps.rearrange("p (h d) -> p h d", h=NH))
                nc.sync.dma_start(out=vA[sp][:, :, DK:DK + 1], in_=ones_h)

        def emit_qk(fc):
            for x_r, w_sb, dst, bias_sb in ((xq_r, wq_sb, qT, bq_sb),
                                            (xk_r, wk_sb, kT, None)):
                for sf in range(NSF):
                    ps = pj_pool.tile([P, SF], F32, tag="ps")
                    for co in range(NCO):
                        nc.tensor.matmul(
                            ps, w_sb[:, co, fc * P:(fc + 1) * P],
                            x_r[:, co, sf * SF:(sf + 1) * SF],
                            start=(co == 0), stop=(co == NCO - 1))
                    dst_ap = dst[fc][:, sf * SF:(sf + 1) * SF]
                    if bias_sb is not None:
                        nc.vector.tensor_scalar_add(dst_ap, ps,
                                                    bias_sb[:, fc:fc + 1])
                    else:
                        nc.vector.tensor_copy(out=dst_ap, in_=ps)

        for fc in range(NFP):
            emit_qk(fc)
            for qi in range(NSF):
                if fc == 0:
                    emit_vproj(qi)
                qv = {o: qT[fc][o * DK:(o + 1) * DK, qi * SF:(qi + 1) * SF]
                      for o in (0, 1)}
                po = {o: op_pool.tile([P, SF], F32, tag="po", name=f"po{o}")
                      for o in (0, 1)}
                n_used = len(used[qi])
                for i, jj in enumerate(used[qi]):
                    t0 = trim(qi, jj)
                    # both heads' scoresT for this key chunk share one 2-bank
                    # psum tile; the even/odd matmuls use disjoint 64-row
                    # groups of the PE array and run concurrently.
                    pse = sp_pool.tile([P, 2 * SF], F32, tag="ps")
                    for o in (0, 1):
                        nc.tensor.matmul(
                            pse[:, o * SF + t0:(o + 1) * SF],
                            kT[fc][o * DK:(o + 1) * DK, jj * P:(jj + 1) * P],
                            qv[o][:, t0:], start=True, stop=True)
                    et = ep.tile([P, 2 * SF], F16, tag="et")
                    pse3 = pse.rearrange("p (h q) -> p h q", h=2)
                    et3 = et.rearrange("p (h q) -> p h q", h=2)
                    nc.scalar.activation(out=et3[:, :, t0:],
                                         in_=pse3[:, :, t0:], func=AF.Exp)
                    cl = classes[(qi, jj)]
                    if cl == "free":
                        pass
                    elif isinstance(cl, tuple) and cl[0] == "affine":
                        # one gpsimd select covers both head halves (the head
                        # dim gets affine coefficient 0). It runs over the
                        # same trimmed columns as the exp; the fully-masked
                        # columns below t0 are never read by the attnV matmul.
                        nc.gpsimd.affine_select(
                            out=et3[:, :, t0:], in_=et3[:, :, t0:],
                            compare_op=ALU.is_ge, fill=0.0,
                            base=t0 - cl[1], pattern=[[0, 2], [1, SF - t0]],
                            channel_multiplier=-1)
                    else:
                        for o in (0, 1):
                            eth = et[:, o * SF:(o + 1) * SF]
                            nc.vector.tensor_mul(eth, eth, mask_sb[cl])
                    for o in (0, 1):
                        nc.tensor.matmul(
                            po[o][0:DK + 1, t0:], vA[jj][:, 2 * fc + o, :],
                            et[:, o * SF + t0:(o + 1) * SF],
                            start=(i == 0), stop=(i == n_used - 1))
                for o in (0, 1):
                    # denominator lives on partition DK(=64); broadcast its
                    # reciprocal over 64 partitions with a K=1 matmul.
                    dn = dnp.tile([P, SF], F32R, tag="dn")
                    nc.scalar.copy(out=dn[DK:DK + 1, :], in_=po[o][DK:DK + 1, :])
                    rp = rp_pool.tile([DK, SF], F32, tag="rp")
                    nc.tensor.matmul(rp, ones_sb[DK:DK + 1, 0:DK],
                                     dn[DK:DK + 1, :], start=True, stop=True)
                    rc = rcp.tile([DK, SF], F32, tag="rc")
                    nc.vector.reciprocal(rc, rp)
                    if o == 0:
                        nc.vector.tensor_tensor(
                            out=cT[fc][0:DK, qi * SF:(qi + 1) * SF],
                            in0=po[o][0:DK, :], in1=rc, op=ALU.mult)
                    else:
                        stg = stgp.tile([DK, SF], F16, tag="stg")
                        nc.vector.tensor_tensor(out=stg, in0=po[o][0:DK, :],
                                                in1=rc, op=ALU.mult)
                        nc.sync.dma_start(
                            out=cT[fc][DK:2 * DK, qi * SF:(qi + 1) * SF],
                            in_=stg)

    # ---------- Phase 3: output projection ----------
    with tc.tile_pool(name="w_o", bufs=1) as wp, \
         tc.tile_pool(name="ps_o", bufs=4, space="PSUM") as pp, \
         tc.tile_pool(name="ot", bufs=4) as otp:
        wo_sb = wp.tile([P, NFP, D], F16, tag="w")
        nc.sync.dma_start(out=wo_sb, in_=wo.rearrange("(fc p) e -> p fc e", p=P))
        for ec in range(NEC):
            for sc in range(NSF):
                ps = pp.tile([P, SF], F32, tag="ps")
                for fc in range(NFP):
                    nc.tensor.matmul(ps, wo_sb[:, fc, ec * P:(ec + 1) * P],
                                     cT[fc][:, sc * SF:(sc + 1) * SF],
                                     start=(fc == 0), stop=(fc == NFP - 1))
                ot = otp.tile([P, SF], F16, tag="ot")
                nc.vector.tensor_copy(out=ot, in_=ps)
                nc.sync.dma_start(
                    out=outT[ec * P:(ec + 1) * P, sc * SF:(sc + 1) * SF], in_=ot)


def build(classes, n_pat, reps=1):
    nc = bacc.Bacc("TRN2", target_bir_lowering=False, debug=False)

    def din(name, shape, dt=F32R):
        return nc.dram_tensor(name, shape, dt, kind="ExternalInput").ap()

    xq, xk, xv = (din("xqT", (D, S), F16), din("xkT", (D, S), F16),
                  din("xvT", (D, S), F16))
    wq, wk, wv = (din("wqT", (D, F), F16), din("wkT", (D, F), F16),
                  din("wvT", (D, F), F16))
    wo = din("woT", (F, D), F16)
    bq = din("bq", (F,), F32)
    mm = din("mmix", (max(n_pat, 1), P, SF), F16)
    ones_d = din("ones_d", (P, DK))
    ones_h = din("ones_h", (P, NH), F16)
    outT = nc.dram_tensor("outT", (D, S), F16, kind="ExternalOutput").ap()

    with tile.TileContext(nc) as tc:
        with ExitStack() as ctx:
            if reps == 1:
                _body(ctx, tc, xq, xk, xv, wq, wk, wv, wo, bq, mm, ones_d,
                      ones_h, outT, classes, n_pat)
            else:
                with tc.For_i(0, reps, 1):
                    _body(ctx, tc, xq, xk, xv, wq, wk, wv, wo, bq, mm, ones_d,
                          ones_h, outT, classes, n_pat)
    nc.compile()
    return nc


def classify_mask(mask2d):
    """Per (qi, jj) chunk of the [S, S] bool mask: 'skip' (all False),
    'free' (all True), ('affine', off) for causal-style chunks
    (valid iff q >= j), or a dedup'd mixed-pattern id (stored transposed
    [keys, queries] as f32 multiplicative masks)."""
    classes = {}
    patterns = []
    pattern_keys = {}
    ql = np.arange(SF)[:, None]
    jl = np.arange(P)[None, :]
    for qi in range(NSF):
        for jj in range(NJJ):
            chunk = mask2d[qi * SF:(qi + 1) * SF, jj * P:(jj + 1) * P]
            if not chunk.any():
                classes[(qi, jj)] = "skip"
            elif chunk.all():
                classes[(qi, jj)] = "free"
            else:
                off = 128 * jj - 512 * qi
                if np.array_equal(chunk, ql >= jl + off):
                    classes[(qi, jj)] = ("affine", off)
                    continue
                key = chunk.tobytes()
                if key not in pattern_keys:
                    pattern_keys[key] = len(patterns)
                    patterns.append(np.ascontiguousarray(chunk.T).astype(np.float32))
                classes[(qi, jj)] = pattern_keys[key]
    return classes, patterns


def _head_index(g):
    # local feature f = hl*64 + d  maps to reference row  d*16 + (8g + hl)
    hl = np.arange(NH)
    d = np.arange(DK)
    return (d[None, :] * H + (NH * g + hl)[:, None]).reshape(-1)


def make_in_maps(query, key, value, w_q, b_q, w_k, w_v, w_o, patterns):
    n_pat = max(len(patterns), 1)
    mm = np.zeros((n_pat, P, SF), np.float32)
    for i, pat in enumerate(patterns):
        mm[i] = pat
    in_maps = []
    for c in range(8):
        b, g = divmod(c, 2)
        idx = _head_index(g)
        in_maps.append({
            "xqT": np.ascontiguousarray(query[b].T).astype(np.float16),
            "xkT": np.ascontiguousarray(key[b].T).astype(np.float16),
            "xvT": np.ascontiguousarray(value[b].T).astype(np.float16),
            # fold the 1/sqrt(dk) score scaling into Wq (exact: power of two)
            "wqT": (np.ascontiguousarray(w_q[idx, :].T) * SCALE).astype(np.float16),
            "wkT": np.ascontiguousarray(w_k[idx, :].T).astype(np.float16),
            "wvT": np.ascontiguousarray(w_v[idx, :].T).astype(np.float16),
            "woT": np.ascontiguousarray(w_o[:, F * g:F * (g + 1)].T).astype(np.float16),
            "bq": (b_q[idx] * SCALE).astype(np.float32),
            "mmix": mm.astype(np.float16),
            "ones_d": np.ones((P, DK), np.float32),
            "ones_h": np.ones((P, NH), np.float16),
        })
    return in_maps


def fold_output_bias(b_o, b_v, w_o):
    # softmax rows sum to 1 => v-bias contributes  w_o @ bv_concat  exactly.
    bv_concat = np.zeros(D, np.float32)
    for g in range(2):
        bv_concat[F * g:F * (g + 1)] = b_v[_head_index(g)]
    return (b_o + w_o @ bv_concat).astype(np.float32)


def _reference_numpy(query, key, value, mask, w_q, b_q, w_k, b_k, w_v, b_v,
                     w_o, b_o):
    # exact fallback mirroring reference.py (chunked per batch*head)
    Bn, Sn, Dn = query.shape
    Hn = H
    DKn = Dn // Hn
    q = query @ w_q.T + b_q
    k = key @ w_k.T + b_k
    v = value @ w_v.T + b_v

    def split(x):
        return x.reshape(Bn, Sn, DKn, Hn).transpose(0, 3, 1, 2)

    q, k, v = split(q), split(k), split(v)
    m = np.broadcast_to(np.asarray(mask), (1, 1, Sn, Sn))[0, 0]
    out = np.empty((Bn, Sn, Hn * DKn), np.float32)
    scale = 1.0 / np.sqrt(DKn)
    for b in range(Bn):
        for h in range(Hn):
            s = (q[b, h] @ k[b, h].T) * scale
            s = np.where(m, s, -np.inf)
            s -= s.max(axis=-1, keepdims=True)
            e = np.exp(s)
            p = e / e.sum(axis=-1, keepdims=True)
            out[b, :, h * DKn:(h + 1) * DKn] = p @ v[b, h]
    return out @ w_o.T + b_o


_CACHE = {}

# test harness hooks: set TRACE=True before calling kernel() to profile;
# the raw BassKernelResults of the last run lands in LAST_RESULTS.
TRACE = False
LAST_RESULTS = None


def kernel(query, key, value, mask, w_q, b_q, w_k, b_k, w_v, b_v, w_o, b_o):
    query = np.asarray(query, np.float32)
    key = np.asarray(key, np.float32)
    value = np.asarray(value, np.float32)
    shapes_ok = (query.shape == (B, S, D) and key.shape == (B, S, D)
                 and value.shape == (B, S, D)
                 and np.asarray(mask).shape[-2:] == (S, S)
                 and w_q.shape == (D, D) and w_o.shape == (D, D))
    if not shapes_ok:
        return _reference_numpy(query, key, value, mask, w_q, b_q, w_k, b_k,
                                w_v, b_v, w_o, b_o)

    mask2d = np.broadcast_to(np.asarray(mask), (1, 1, S, S))[0, 0].astype(bool)
    classes, patterns = classify_mask(mask2d)
    if len(patterns) > MAX_MASK_PATTERNS or any(
            all(classes[(qi, jj)] == "skip" for jj in range(NJJ))
            for qi in range(NSF)):
        return _reference_numpy(query, key, value, mask, w_q, b_q, w_k, b_k,
                                w_v, b_v, w_o, b_o)

    ckey = tuple(sorted(classes.items(), key=repr)) + (len(patterns),)
    if ckey not in _CACHE:
        _CACHE[ckey] = build(classes, len(patterns))
    nc = _CACHE[ckey]

    in_maps = make_in_maps(query, key, value,
                           np.asarray(w_q, np.float32), np.asarray(b_q, np.float32),
                           np.asarray(w_k, np.float32), np.asarray(w_v, np.float32),
                           np.asarray(w_o, np.float32), patterns)
    res = run_bass_kernel_spmd(nc, in_maps, core_ids=list(range(8)),
                               trace=TRACE)
    global LAST_RESULTS
    LAST_RESULTS = res

    bo_eff = fold_output_bias(np.asarray(b_o, np.float32),
                              np.asarray(b_v, np.float32),
                              np.asarray(w_o, np.float32))
    out = np.empty((B, S, D), np.float32)
    for b in range(B):
        acc = (res.results[2 * b]["outT"].astype(np.float32).T
               + res.results[2 * b + 1]["outT"].astype(np.float32).T)
        out[b] = acc + bo_eff
    return out

